# revision 1
# baseline (speedup 1.0000x reference)
"""Trainium2 Bass kernel for nn_EntRelJointDecoder_68212670595943.

Computes element_loss + q_loss (scalar f32) of the reference EntRelJointDecoder:
  - joint CE over joint_score [B,S,S,V]
  - CE over softmax(q_score) for the quintuplet tensor [B,S,S,S,O]

Sharding: 8 cores = (batch b in 0..3) x (x-half in 0..1). Each core handles
q_score[b, xh*48:(xh+1)*48, :, :, :] and the matching joint slice, reducing
everything on-chip to 6 partial sums; the host combines partials.

Math used on-device (per core, XY = 48*96 = 4608 pair rows):
  pair[xy, i]  = gelu(A[x] + C[y] + pair_b),  A = x@W1, C = x@W2 (pair_W split)
  q^T[zo, xy]  = sum_i uv[zo, i] * pair[xy, i]           (PE, bf16, fp32 acc)
  e = exp(q);  s[z, xy] = sum_o e  (PE matmul with 0/1 group matrix G)
  r = 1/s;  p = e * broadcast(r);  ep = exp(p)
  sp[z, xy] = sum_o ep (PE);  lp = ln(sp)
  q_loss numer = sum lp*mask - sum p*Wq   (Wq = one-hot(label)*mask, host-built)
  joint: js^T[v, xy] = pair@final_W + b; lse = ln(sum_v exp(js)); minus js[label]
"""

import numpy as np

try:
    import ml_dtypes

    BF16 = ml_dtypes.bfloat16
except ImportError:  # pragma: no cover
    BF16 = None

B, S, H, M, V, O = 4, 96, 768, 256, 20, 20
NCORES = 8
XL = S // 2  # 48 x rows per core
XY = XL * S  # 4608 pair rows per core
ZO = S * O  # 1920 (z,o) rows
ZT = 120  # zo rows per tile (6 z groups of 20)
NZT = ZO // ZT  # 16
ZPT = ZT // O  # 6 z per zo tile
WST = 512  # xy stripe width (one PSUM bank of f32)
NST = XY // WST  # 9 stripes
TP = 2  # zo-tiles merged per q/e tile
NTP = NZT // TP  # 8
KT = M // 128  # 2 contraction tiles over i
HKT = H // 128  # 6 contraction tiles over h

# How many of the per-(tp,stripe) B-dot ops run on GPSIMD (rest on VectorE).
N_BDOT_GPSIMD_FRAC = 0.0

_PROGRAM_CACHE = {}


def _build_program():
    import os
    from contextlib import ExitStack

    disable = set(os.environ.get("KERNEL_DISABLE", "").split(","))

    import concourse.bacc as bacc
    import concourse.bass as bass
    from concourse import mybir
    from concourse.tile import TileContext

    dt = mybir.dt
    AF = mybir.ActivationFunctionType
    ALU = mybir.AluOpType

    nc = bacc.Bacc()

    xT = nc.declare_dram_parameter("xT", [H, S], dt.bfloat16, isOutput=False)
    xTh = nc.declare_dram_parameter("xTh", [H, XL], dt.bfloat16, isOutput=False)
    w1 = nc.declare_dram_parameter("w1", [H, M], dt.bfloat16, isOutput=False)
    w2 = nc.declare_dram_parameter("w2", [H, M], dt.bfloat16, isOutput=False)
    vw = nc.declare_dram_parameter("vw", [H, M], dt.bfloat16, isOutput=False)
    fw = nc.declare_dram_parameter("fw", [M, V], dt.bfloat16, isOutput=False)
    pb = nc.declare_dram_parameter("pb", [M, 1], dt.float32, isOutput=False)
    vb = nc.declare_dram_parameter("vb", [M, 1], dt.float32, isOutput=False)
    fb = nc.declare_dram_parameter("fb", [V, 1], dt.float32, isOutput=False)
    ut = nc.declare_dram_parameter("ut", [O, M, M], dt.bfloat16, isOutput=False)
    gm = nc.declare_dram_parameter("gm", [ZT, NZT * S], dt.bfloat16, isOutput=False)
    wq = nc.declare_dram_parameter(
        "wq", [ZT, (NTP // 2) * NST * 2 * TP * WST], dt.bfloat16, isOutput=False
    )
    wj = nc.declare_dram_parameter("wj", [V, XY], dt.bfloat16, isOutput=False)
    qm = nc.declare_dram_parameter("qm", [S, XY], dt.bfloat16, isOutput=False)
    jm = nc.declare_dram_parameter("jm", [1, XY], dt.bfloat16, isOutput=False)
    onesp = nc.declare_dram_parameter("onesp", [128, 1], dt.float32, isOutput=False)
    ex = nc.declare_dram_parameter("ex", [XL, XY], dt.bfloat16, isOutput=False)
    ey = nc.declare_dram_parameter("ey", [S, XY], dt.bfloat16, isOutput=False)
    pbr = nc.declare_dram_parameter("pbr", [1, M], dt.bfloat16, isOutput=False)
    ones48 = nc.declare_dram_parameter("ones48", [1, XL], dt.bfloat16, isOutput=False)
    ones20 = nc.declare_dram_parameter("ones20", [V, 1], dt.bfloat16, isOutput=False)
    partials = nc.declare_dram_parameter("partials", [8, 1], dt.float32, isOutput=True)

    n_bdot_gp = int(round(N_BDOT_GPSIMD_FRAC * (NTP // 2) * NST))

    with TileContext(nc) as tc, ExitStack() as ctx:
        consts = ctx.enter_context(tc.tile_pool(name="consts", bufs=1))
        work = ctx.enter_context(tc.tile_pool(name="work", bufs=1))
        epool = ctx.enter_context(tc.tile_pool(name="epool", bufs=2))
        ppool = ctx.enter_context(tc.tile_pool(name="ppool", bufs=3))
        dmapool = ctx.enter_context(tc.tile_pool(name="dmapool", bufs=3))
        small = ctx.enter_context(tc.tile_pool(name="small", bufs=2))
        upool = ctx.enter_context(tc.tile_pool(name="upool", bufs=3))
        big_ps = ctx.enter_context(tc.tile_pool(name="big_ps", bufs=3, space="PSUM"))
        acc_ps = ctx.enter_context(tc.tile_pool(name="acc_ps", bufs=2, space="PSUM"))
        dram = ctx.enter_context(tc.tile_pool(name="dram", bufs=2, space="DRAM"))

        # ---------------- constants / weights to SBUF ----------------
        w1sb = consts.tile([128, HKT, M], dt.bfloat16)
        w2sb = consts.tile([128, HKT, M], dt.bfloat16)
        vwsb = consts.tile([128, HKT, M], dt.bfloat16)
        xtsb = consts.tile([128, HKT, S], dt.bfloat16)
        xthsb = consts.tile([128, HKT, XL], dt.bfloat16)
        for k in range(HKT):
            nc.sync.dma_start(out=w1sb[:, k, :], in_=w1[k * 128 : (k + 1) * 128, :])
            nc.sync.dma_start(out=w2sb[:, k, :], in_=w2[k * 128 : (k + 1) * 128, :])
            nc.sync.dma_start(out=vwsb[:, k, :], in_=vw[k * 128 : (k + 1) * 128, :])
            nc.sync.dma_start(out=xtsb[:, k, :], in_=xT[k * 128 : (k + 1) * 128, :])
            nc.sync.dma_start(out=xthsb[:, k, :], in_=xTh[k * 128 : (k + 1) * 128, :])
        fwsb = consts.tile([128, KT, V], dt.bfloat16)
        pbsb = consts.tile([128, KT, 1], dt.float32)
        vbsb = consts.tile([128, KT, 1], dt.float32)
        for k in range(KT):
            nc.sync.dma_start(out=fwsb[:, k, :], in_=fw[k * 128 : (k + 1) * 128, :])
            nc.sync.dma_start(out=pbsb[:, k, :], in_=pb[k * 128 : (k + 1) * 128, :])
            nc.sync.dma_start(out=vbsb[:, k, :], in_=vb[k * 128 : (k + 1) * 128, :])
        onespsb = consts.tile([128, 1], dt.float32)
        nc.sync.dma_start(out=onespsb, in_=onesp[:, :])
        exsb = consts.tile([XL, XY], dt.bfloat16)
        nc.sync.dma_start(out=exsb, in_=ex[:, :])
        eysb = consts.tile([S, XY], dt.bfloat16)
        nc.sync.dma_start(out=eysb, in_=ey[:, :])
        pbrsb = consts.tile([1, M], dt.bfloat16)
        nc.sync.dma_start(out=pbrsb, in_=pbr[:, :])
        ones48sb = consts.tile([1, XL], dt.bfloat16)
        nc.sync.dma_start(out=ones48sb, in_=ones48[:, :])
        ones20sb = consts.tile([V, 1], dt.bfloat16)
        nc.sync.dma_start(out=ones20sb, in_=ones20[:, :])

        # ---------------- prelude: A^T, C^T, value^T, pairT ----------------
        # ATt[x, i] = x_half @ W1, CTt[y, i] = x @ W2 (row-major layouts so the
        # pair broadcast-sum becomes accumulating PE matmuls vs indicators).
        atbt = work.tile([XL, M], dt.bfloat16)
        ctbt = work.tile([S, M], dt.bfloat16)
        valsb = work.tile([128, KT, S], dt.bfloat16)  # value^T (gelu'ed)
        at_ps = big_ps.tile([XL, M], dt.float32, tag="bigps")
        for k in range(HKT):
            nc.tensor.matmul(
                at_ps, xthsb[:, k, :], w1sb[:, k, :], start=(k == 0), stop=False
            )
        nc.tensor.matmul(at_ps, ones48sb, pbrsb, start=False, stop=True)
        nc.vector.tensor_copy(out=atbt, in_=at_ps)
        ct_ps = big_ps.tile([S, M], dt.float32, tag="bigps")
        for k in range(HKT):
            nc.tensor.matmul(
                ct_ps, xtsb[:, k, :], w2sb[:, k, :], start=(k == 0), stop=(k == HKT - 1)
            )
        nc.vector.tensor_copy(out=ctbt, in_=ct_ps)
        for it in range(KT):
            isl = slice(it * 128, (it + 1) * 128)
            v_ps = big_ps.tile([128, S], dt.float32, tag="bigps")
            for k in range(HKT):
                nc.tensor.matmul(
                    v_ps, vwsb[:, k, isl], xtsb[:, k, :], start=(k == 0), stop=(k == HKT - 1)
                )
            nc.scalar.activation(out=valsb[:, it, :], in_=v_ps, func=AF.Gelu, bias=vbsb[:, it, :])

        # pairT[i, xl*96+y] = gelu(ATt[xl, i] + CTt[y, i] + pair_b[i]) via
        # three accumulating matmuls against indicator matrices.
        pairT = work.tile([128, KT, XY], dt.bfloat16)
        for it in range(KT):
            isl = slice(it * 128, (it + 1) * 128)
            for ch in range(NST):
                ccols = slice(ch * WST, (ch + 1) * WST)
                pp_ps = big_ps.tile([128, WST], dt.float32, tag="bigps")
                nc.tensor.matmul(
                    pp_ps, atbt[:, isl], exsb[:, ccols], start=True, stop=False
                )
                nc.tensor.matmul(
                    pp_ps, ctbt[:, isl], eysb[:, ccols], start=False, stop=True
                )
                nc.scalar.activation(
                    out=pairT[:, it, ccols], in_=pp_ps, func=AF.Gelu
                )

        # ---------------- uv^T[i, z*20+o] ----------------
        uvT = work.tile([128, KT, ZO], dt.bfloat16)
        uvT4 = uvT.rearrange("p k (z o) -> p k z o", o=O)
        for o in range(O):
            utsb = upool.tile([128, KT, M], dt.bfloat16, tag="ut")
            for jt in range(KT):
                nc.sync.dma_start(out=utsb[:, jt, :], in_=ut[o, jt * 128 : (jt + 1) * 128, :])
            for it in range(KT):
                u_ps = big_ps.tile([128, S], dt.float32, tag="bigps")
                for jt in range(KT):
                    nc.tensor.matmul(
                        u_ps,
                        utsb[:, jt, it * 128 : (it + 1) * 128],
                        valsb[:, jt, :],
                        start=(jt == 0),
                        stop=(jt == KT - 1),
                    )
                nc.vector.tensor_copy(out=uvT4[:, it, :, o], in_=u_ps)

        fbsb = consts.tile([V, 1], dt.float32)
        nc.sync.dma_start(out=fbsb, in_=fb[:, :])
        gsb3 = consts.tile([ZT, NZT * S], dt.bfloat16)
        nc.sync.dma_start(out=gsb3, in_=gm[:, :])
        gsb = gsb3.rearrange("p (t s) -> p t s", s=S)
        qmsb = consts.tile([S, XY], dt.bfloat16)
        nc.sync.dma_start(out=qmsb, in_=qm[:, :])
        jmsb = consts.tile([1, XY], dt.bfloat16)
        nc.sync.dma_start(out=jmsb, in_=jm[:, :])
        m20sb = consts.tile([128, 1], dt.float32)
        nc.vector.memset(m20sb, -20.0)
        p20sb = consts.tile([128, 1], dt.float32)
        nc.vector.memset(p20sb, 20.0)

        # ---------------- accumulators ----------------
        NLC = 3
        lw = XY // NLC
        bcoll = work.tile([ZT, (NTP // 2) * NST], dt.float32)  # sum p*Wq
        lpacc = work.tile([S, 3], dt.float32)  # sum lp*mask (3 chunks)
        elacc_n = work.tile([1, NLC], dt.float32)  # sum lse*mask per chunk
        ejacc = work.tile([V, NST], dt.float32)  # sum js*Wj per stripe
        junk_d = work.tile([ZT, 2 * TP * WST], dt.bfloat16)  # STT dump (DVE)
        junk_g = work.tile([ZT, TP * WST], dt.bfloat16)  # STT dump (GPSIMD)
        junk_j2 = work.tile([V, WST], dt.float32)
        junk_sx = work.tile([S, XY // 3], dt.bfloat16)
        # ln(sum exp) inputs staged so all Ln ops run in one batch at the end
        # (avoids ACT table-set thrash between Exp and Ln).
        spstage = work.tile([S, XY], dt.bfloat16)
        jstage = work.tile([1, XY], dt.float32)
        if disable & {"ttr", "stt"}:
            for acc in (bcoll, lpacc, elacc, ejacc):
                nc.vector.memset(acc, 0.0)

        wq_r = wq.rearrange("p (g s w) -> p g s w", g=NTP // 2, s=NST)

        # ---------------- main loop over xy stripes (sw-pipelined) ----------------
        def phase1(st):
            cols = slice(st * WST, (st + 1) * WST)
            # q = pair.uv, e = exp(q), s = sum_o e
            s_ps = acc_ps.tile([S, WST], dt.float32, tag="accps", name=f"s_ps{st}")
            e_tiles = []
            for tp in range(NTP):
                q_ps = big_ps.tile(
                    [ZT, TP * WST], dt.float32, tag="bigps", name=f"q_ps{st}_{tp}"
                )
                for h in range(TP):
                    t = TP * tp + h
                    zsl = slice(t * ZT, (t + 1) * ZT)
                    for k in range(KT):
                        nc.tensor.matmul(
                            q_ps[:, h * WST : (h + 1) * WST],
                            uvT[:, k, zsl],
                            pairT[:, k, cols],
                            start=(k == 0),
                            stop=(k == KT - 1),
                        )
                e2 = epool.tile(
                    [ZT, TP * WST], dt.bfloat16, tag=f"e{tp}", name=f"e{st}_{tp}", bufs=3
                )
                nc.scalar.activation(out=e2, in_=q_ps, func=AF.Exp)
                e_tiles.append(e2)
                for h in range(TP):
                    t = TP * tp + h
                    nc.tensor.matmul(
                        s_ps,
                        gsb[:, t, :],
                        e2[:, h * WST : (h + 1) * WST],
                        start=(t == 0),
                        stop=(t == NZT - 1),
                    )

            # r = 1/s, staged to DRAM for partition-broadcast reload
            rsb = small.tile([S, WST], dt.float32, tag="rsb", name=f"rsb{st}", bufs=1)
            if "recip" in disable:
                nc.vector.reciprocal(out=rsb, in_=s_ps)
            else:
                nc.vector.reciprocal_approx_fast(out=rsb, in_=s_ps)
            rbf = small.tile([S, WST], dt.bfloat16, tag="rbf", name=f"rbf{st}")
            nc.vector.tensor_copy(out=rbf, in_=rsb)
            rscr = dram.tile([S, WST], dt.bfloat16, tag="rscr", name=f"rscr{st}")
            nc.gpsimd.dma_start(out=rscr, in_=rbf)
            return e_tiles, rscr

        def phase2(st, e_tiles, rscr):
            # p = e*r, ep = exp(p), sp = sum_o ep, B-dot (two tp merged per op)
            sp_ps = acc_ps.tile([S, WST], dt.float32, tag="accps", name=f"sp_ps{st}")
            W2 = TP * WST
            for g in range(NTP // 2):
                rex = dmapool.tile([ZT, 2 * W2], dt.bfloat16, tag="rex", bufs=2)
                if "rex" in disable:
                    nc.vector.memset(rex, 0.05)
                else:
                    for h in range(2 * TP):
                        rex_src = bass.AP(
                            tensor=rscr.tensor,
                            offset=rscr.offset + (2 * TP * g + h) * ZPT * WST,
                            ap=[[WST, ZPT], [0, O], [1, WST]],
                        )
                        nc.gpsimd.dma_start(
                            out=rex[:, h * WST : (h + 1) * WST], in_=rex_src
                        )
                wqt = dmapool.tile([ZT, 2 * W2], dt.bfloat16, tag="wqt", bufs=2)
                if "wqdma" in disable:
                    nc.vector.memset(wqt, 0.0)
                else:
                    nc.sync.dma_start(out=wqt, in_=wq_r[:, g, st, :])
                p2 = ppool.tile([ZT, 2 * W2], dt.bfloat16, tag="p2", bufs=2)
                for half in range(2):
                    tp = 2 * g + half
                    nc.vector.tensor_mul(
                        p2[:, half * W2 : (half + 1) * W2],
                        e_tiles[tp],
                        rex[:, half * W2 : (half + 1) * W2],
                    )
                ep2 = ppool.tile([ZT, 2 * W2], dt.bfloat16, tag="ep2", bufs=2)
                nc.scalar.activation(out=ep2, in_=p2, func=AF.Exp)
                for h in range(2 * TP):
                    t = 2 * TP * g + h
                    nc.tensor.matmul(
                        sp_ps,
                        gsb[:, t, :],
                        ep2[:, h * WST : (h + 1) * WST],
                        start=(t == 0),
                        stop=(t == NZT - 1),
                    )
                col = g * NST + st
                if "ttr" in disable:
                    pass
                elif col < n_bdot_gp:
                    nc.gpsimd.scalar_tensor_tensor(
                        out=junk_g,
                        in0=p2,
                        scalar=1.0,
                        in1=wqt,
                        op0=ALU.mult,
                        op1=ALU.mult,
                        accum_out=bcoll[:, col : col + 1],
                    )
                else:
                    nc.vector.scalar_tensor_tensor(
                        out=junk_d,
                        in0=p2,
                        scalar=1.0,
                        in1=wqt,
                        op0=ALU.mult,
                        op1=ALU.mult,
                        accum_out=bcoll[:, col : col + 1],
                    )
            cols = slice(st * WST, (st + 1) * WST)

            # stage sp for the deferred Ln batch
            nc.scalar.activation(
                out=spstage[:, cols], in_=sp_ps, func=AF.Identity, bias=m20sb[:S]
            )

            # joint (element) part for this stripe
            js_ps = big_ps.tile([V, WST], dt.float32, tag="bigps", name=f"js_ps{st}")
            for k in range(KT):
                nc.tensor.matmul(
                    js_ps,
                    fwsb[:, k, :],
                    pairT[:, k, cols],
                    start=(k == 0),
                    stop=(k == KT - 1),
                )
            ejs = small.tile([V, WST], dt.bfloat16, tag="ejs", name=f"ejs{st}")
            nc.scalar.activation(out=ejs, in_=js_ps, func=AF.Exp, bias=fbsb)
            sjs_ps = big_ps.tile([1, WST], dt.float32, tag="bigps", name=f"sjs_ps{st}")
            nc.tensor.matmul(sjs_ps, ones20sb, ejs, start=True, stop=True)
            nc.scalar.activation(out=jstage[:, cols], in_=sjs_ps, func=AF.Identity)
            wjt = dmapool.tile([V, WST], dt.bfloat16, tag="wjt", name=f"wjt{st}")
            nc.sync.dma_start(out=wjt, in_=wj[:, cols])
            if "ttr" not in disable:
                # note: reads js WITHOUT final_b; host adds sum(fb[label]*mask)
                nc.vector.scalar_tensor_tensor(
                    out=junk_j2,
                    in0=js_ps,
                    scalar=1.0,
                    in1=wjt,
                    op0=ALU.mult,
                    op1=ALU.mult,
                    accum_out=ejacc[:, st : st + 1],
                )

        def ln_chunk(c):
            # chunk c covers stripes 3c..3c+2; run as soon as those are staged
            csl = slice(c * lw, (c + 1) * lw)
            nc.scalar.activation(
                out=spstage[:, csl], in_=spstage[:, csl], func=AF.Ln, bias=p20sb[:S]
            )
            nc.scalar.activation(
                out=jstage[:, csl], in_=jstage[:, csl], func=AF.Ln
            )
            if "stt" not in disable:
                nc.vector.scalar_tensor_tensor(
                    out=junk_sx,
                    in0=spstage[:, csl],
                    scalar=1.0,
                    in1=qmsb[:, csl],
                    op0=ALU.mult,
                    op1=ALU.mult,
                    accum_out=lpacc[:, c : c + 1],
                )
                nc.vector.scalar_tensor_tensor(
                    out=junk_sx[:1, :],
                    in0=jstage[:, csl],
                    scalar=1.0,
                    in1=jmsb[:, csl],
                    op0=ALU.mult,
                    op1=ALU.mult,
                    accum_out=elacc_n[:, c : c + 1],
                )

        # software pipeline: emit phase1 of stripe k+1 before phase2 of k;
        # deferred-Ln chunks run as soon as their three stripes are staged
        state = {0: phase1(0), 1: phase1(1)}
        for st in range(NST):
            if st + 2 < NST:
                state[st + 2] = phase1(st + 2)
            phase2(st, *state.pop(st))
            if st % 3 == 2:
                ln_chunk(st // 3)

        # ---------------- final reduction to 8 scalars ----------------
        stag = work.tile([128, 8], dt.float32)
        nc.vector.memset(stag, 0.0)
        nc.vector.reduce_sum(
            out=stag[:S, 0:1], in_=lpacc, axis=mybir.AxisListType.X
        )
        nc.vector.reduce_sum(
            out=stag[:ZT, 1:2], in_=bcoll, axis=mybir.AxisListType.X
        )
        nc.vector.reduce_sum(
            out=stag[:S, 2:3], in_=qmsb, axis=mybir.AxisListType.X
        )
        nc.vector.reduce_sum(
            out=stag[:1, 3:4], in_=elacc_n, axis=mybir.AxisListType.X
        )
        nc.vector.reduce_sum(
            out=stag[:V, 4:5], in_=ejacc, axis=mybir.AxisListType.X
        )
        nc.vector.reduce_sum(
            out=stag[:1, 5:6], in_=jmsb, axis=mybir.AxisListType.X
        )
        fin_ps = big_ps.tile([8, 1], dt.float32, tag="bigps")
        nc.tensor.matmul(fin_ps, stag, onespsb, start=True, stop=True)
        outsb = work.tile([8, 1], dt.float32)
        nc.vector.tensor_copy(out=outsb, in_=fin_ps)
        nc.sync.dma_start(out=partials[:, :], in_=outsb)

    nc.compile()
    return nc


def _get_program():
    if "nc" not in _PROGRAM_CACHE:
        _PROGRAM_CACHE["nc"] = _build_program()
    return _PROGRAM_CACHE["nc"]


def _shard_inputs(inputs):
    x = np.asarray(inputs["seq_encoder_reprs"], np.float32)
    pW = np.asarray(inputs["pair_W"], np.float32)
    pb = np.asarray(inputs["pair_b"], np.float32)
    fW = np.asarray(inputs["final_W"], np.float32)
    fb = np.asarray(inputs["final_b"], np.float32)
    vW = np.asarray(inputs["value_W"], np.float32)
    vb = np.asarray(inputs["value_b"], np.float32)
    U = np.asarray(inputs["U"], np.float32)
    jlab = np.asarray(inputs["joint_label_matrix"])
    jmask = np.asarray(inputs["joint_label_matrix_mask"])
    qlab = np.asarray(inputs["quintuplet_matrix"])
    qmask = np.asarray(inputs["quintuplet_matrix_mask"])

    bf = BF16
    shared = {
        "w1": np.ascontiguousarray(pW[:H].astype(bf)),
        "w2": np.ascontiguousarray(pW[H:].astype(bf)),
        "vw": np.ascontiguousarray(vW.astype(bf)),
        "fw": np.ascontiguousarray(fW.astype(bf)),
        "pb": np.ascontiguousarray(pb.reshape(M, 1)),
        "vb": np.ascontiguousarray(vb.reshape(M, 1)),
        "fb": np.ascontiguousarray(fb.reshape(V, 1)),
        "ut": np.ascontiguousarray(U.transpose(0, 2, 1).astype(bf)),
        "onesp": np.ones((128, 1), np.float32),
        "pbr": np.ascontiguousarray(pb.reshape(1, M).astype(bf)),
        "ones48": np.ones((1, XL), bf),
        "ones20": np.ones((V, 1), bf),
        "partials": np.zeros((8, 1), np.float32),
    }
    ex_m = np.zeros((XL, XY), np.float32)
    for xl in range(XL):
        ex_m[xl, xl * S : (xl + 1) * S] = 1.0
    shared["ex"] = ex_m.astype(bf)
    ey_m = np.tile(np.eye(S, dtype=np.float32), (1, XL))
    shared["ey"] = np.ascontiguousarray(ey_m.astype(bf))
    g = np.zeros((NZT, ZT, S), np.float32)
    for t in range(NZT):
        for p_ in range(ZT):
            g[t, p_, ZPT * t + p_ // O] = 1.0
    shared["gm"] = np.ascontiguousarray(
        g.transpose(1, 0, 2).reshape(ZT, NZT * S).astype(bf)
    )

    oidx = np.arange(O, dtype=np.int32)
    vidx = np.arange(V, dtype=np.int32)
    maps = []
    for c in range(NCORES):
        b, xh = divmod(c, 2)
        xsl = slice(xh * XL, (xh + 1) * XL)
        d = dict(shared)
        xb = x[b]
        d["xT"] = np.ascontiguousarray(xb.T.astype(bf))
        d["xTh"] = np.ascontiguousarray(xb[xsl].T.astype(bf))

        ql = qlab[b, xsl]  # [XL, S(y), S(z)] int
        qmk = qmask[b, xsl]  # bool
        labT = ql.transpose(2, 0, 1).reshape(S, XY)
        mT = qmk.transpose(2, 0, 1).reshape(S, XY)
        wq_full = (labT[:, None, :] == oidx[None, :, None]) & mT[:, None, :]
        wqm = wq_full.reshape(ZO, XY)  # [zo, xy]
        # regroup to [ZT, g, st, (h w)] so each merged B-dot slice is one
        # contiguous DMA: zo = (4g+h)*120 + pp, xy = st*WST + w
        wq5 = wqm.reshape(NTP // 2, 2 * TP, ZT, NST, WST)
        wq5 = wq5.transpose(2, 0, 3, 1, 4)  # [ZT, g, st, h, w]
        d["wq"] = np.ascontiguousarray(
            wq5.reshape(ZT, (NTP // 2) * NST * 2 * TP * WST).astype(bf)
        )
        d["qm"] = np.ascontiguousarray(mT.astype(bf))

        jl = jlab[b, xsl].reshape(XY)
        jmk = jmask[b, xsl].reshape(XY)
        wj_full = (jl[None, :] == vidx[:, None]) & jmk[None, :]
        d["wj"] = np.ascontiguousarray(wj_full.astype(bf))
        d["jm"] = np.ascontiguousarray(jmk.reshape(1, XY).astype(bf))
        maps.append(d)
    return maps


def _combine(results, jsl_bias_correction):
    tot = np.zeros(8, np.float64)
    for r in results:
        tot += r["partials"].reshape(8).astype(np.float64)
    q_lp, q_pl, q_cnt, e_lse, e_jsl, e_cnt = tot[:6]
    e_jsl += jsl_bias_correction
    loss = (e_lse - e_jsl) / e_cnt + (q_lp - q_pl) / q_cnt
    return np.float32(loss)


def _jsl_bias_correction(inputs):
    """sum over all masked joint positions of final_b[label] (folded on host
    because the device B-dot reads js before the bias add)."""
    fb = np.asarray(inputs["final_b"], np.float64)
    jl = np.asarray(inputs["joint_label_matrix"]).astype(np.int64)
    jmk = np.asarray(inputs["joint_label_matrix_mask"]).astype(np.float64)
    return float((fb[jl] * jmk).sum())


def kernel(**inputs):
    from concourse.bass_utils import run_bass_kernel_spmd

    nc = _get_program()
    in_maps = _shard_inputs(inputs)
    res = run_bass_kernel_spmd(nc, in_maps, list(range(NCORES)))
    return _combine(res.results, _jsl_bias_correction(inputs))


def kernel_traced(**inputs):
    """Like kernel() but with NTFF tracing; returns (output, BassKernelResults)."""
    from concourse.bass_utils import run_bass_kernel_spmd

    nc = _get_program()
    in_maps = _shard_inputs(inputs)
    res = run_bass_kernel_spmd(
        nc, in_maps, list(range(NCORES)), trace=True
    )
    return _combine(res.results, _jsl_bias_correction(inputs)), res

